# revision 19
# baseline (speedup 1.0000x reference)
"""MoE-LoRA with gumbel straight-through routing on 8 TRN2 NeuronCores.

gates = y_hard + y_soft - stop_grad(y_soft) is numerically exactly
one-hot, so only the argmax expert per token contributes.  Per-core
(512 tokens, data-parallel over B): load x f32 -> cast fp16 ->
PE-transpose planes -> gating matmuls (fp16 stream vs split gw, sigma
and gw-norm folded in) + Gram diag for ||x|| -> gumbel+argmax ->
routing tables via compare/prefix matmuls -> dma_gather(transpose=True)
builds expert-sorted xT and per-slot down weights -> static down
matmuls -> up matmuls with register-offset expert rhs -> indirect
scatter of output token rows.
"""
import sys
sys.path.insert(0, "/opt/trn_rl_repo")
import numpy as np

import concourse.bass as bass
import concourse.mybir as mybir
import concourse.tile as tile
from concourse import bacc
from concourse.bass_utils import run_bass_kernel_spmd
from concourse.masks import make_identity

F32 = mybir.dt.float32
F16 = mybir.dt.float16
I16 = mybir.dt.int16
I32 = mybir.dt.int32
U32 = mybir.dt.uint32
AX = mybir.AxisListType
OP = mybir.AluOpType
ACTF = mybir.ActivationFunctionType

NCORE = 8
B, F_, H, N, R = 4096, 16, 1280, 8, 64
BC = B // NCORE            # tokens per core = 512
ST = 128                   # tokens per subtile
NSUB = BC // ST            # 4
NCH = H // 128             # 10 h-chunks
C = F_ * H                 # 20480
NBLK = ST * F_ // 128      # 16 natural blocks per subtile
NSLOT = 23                 # static 8-token slots per subtile (>= 16+7 worst case)
NQ = NSLOT * 8             # sorted token positions incl. pad = 192
EPS = 1e-12
BIGROW = 60000.0           # scatter skip marker (> BC*F_-1)


def build_nc():
    nc = bacc.Bacc("TRN2", target_bir_lowering=False, debug=False)
    x = nc.dram_tensor("x", [BC * F_, H], F32, kind="ExternalInput").ap()
    u = nc.dram_tensor("u", [BC, N], F32, kind="ExternalInput").ap()
    gw = nc.dram_tensor("gate_w", [N, C], F32, kind="ExternalInput").ap()
    sigma = nc.dram_tensor("sigma", [1, 1], F32, kind="ExternalInput").ap()
    dw = nc.dram_tensor("down_w", [N * R, H], F32, kind="ExternalInput").ap()
    uw = nc.dram_tensor("up_w", [N * H, R], F32, kind="ExternalInput").ap()
    out = nc.dram_tensor("out", [BC * F_, H], F32, kind="ExternalOutput").ap()

    with tile.TileContext(nc) as tc:
        with (
            tc.tile_pool(name="const", bufs=1) as cp,
            tc.tile_pool(name="wts", bufs=1) as wp,
            tc.tile_pool(name="natf32", bufs=2) as natp,
            tc.tile_pool(name="nat16", bufs=1) as nat16p,
            tc.tile_pool(name="planes", bufs=2) as planep,
            tc.tile_pool(name="sorted", bufs=1) as sortp,
            tc.tile_pool(name="small", bufs=2) as sp,
            tc.tile_pool(name="mids", bufs=1) as midp,
            tc.tile_pool(name="outs", bufs=2) as outp,
        ):
            # ================= constants =================
            identf = cp.tile([128, 128], F32)
            make_identity(nc, identf[:])
            identh = cp.tile([128, 128], F16)
            nc.scalar.copy(identh[:], identf[:])
            ident16 = cp.tile([16, 16], F32)
            make_identity(nc, ident16[:])

            diagmask = cp.tile([128, 128], F32)   # 1 on diag else 0
            nc.gpsimd.memset(diagmask[:], 1.0)
            nc.gpsimd.affine_select(out=diagmask[:], in_=diagmask[:],
                                    compare_op=OP.is_ge, fill=0.0,
                                    base=0, pattern=[[-1, 128]], channel_multiplier=1)
            nc.gpsimd.affine_select(out=diagmask[:], in_=diagmask[:],
                                    compare_op=OP.is_ge, fill=0.0,
                                    base=0, pattern=[[1, 128]], channel_multiplier=-1)
            tril128 = cp.tile([128, 128], F32)    # [s, t] = 1 if s < t
            nc.gpsimd.memset(tril128[:], 1.0)
            nc.gpsimd.affine_select(out=tril128[:], in_=tril128[:],
                                    compare_op=OP.is_ge, fill=0.0,
                                    base=-1, pattern=[[1, 128]], channel_multiplier=-1)
            tri8 = cp.tile([8, 8], F32)           # [k, m] = 1 if k < m
            nc.gpsimd.memset(tri8[:], 1.0)
            nc.gpsimd.affine_select(out=tri8[:], in_=tri8[:],
                                    compare_op=OP.is_ge, fill=0.0,
                                    base=-1, pattern=[[1, 8]], channel_multiplier=-1)
            ones128 = cp.tile([128, 1], F32)
            nc.gpsimd.memset(ones128[:], 1.0)
            ones1x32 = cp.tile([1, 32], F32)
            nc.gpsimd.memset(ones1x32[:], 1.0)
            a16 = cp.tile([8, 128], F32)     # a16[q8, p] = 16 iff p//16 == q8
            nc.gpsimd.memset(a16[:], 16.0)
            nc.gpsimd.affine_select(out=a16[:], in_=a16[:], compare_op=OP.is_ge,
                                    fill=0.0, base=0, pattern=[[1, 128]],
                                    channel_multiplier=-16)
            nc.gpsimd.affine_select(out=a16[:], in_=a16[:], compare_op=OP.is_ge,
                                    fill=0.0, base=15, pattern=[[-1, 128]],
                                    channel_multiplier=16)

            _iota_n = [0]
            def iota_f32(shape, pattern, cm=0, base=0):
                _iota_n[0] += 1
                ti = cp.tile(shape, I32, tag=f"iota_i_{_iota_n[0]}")
                nc.gpsimd.iota(ti[:], base=base, pattern=pattern, channel_multiplier=cm)
                tf = cp.tile(shape, F32, tag=f"iota_f_{_iota_n[0]}")
                nc.vector.tensor_copy(tf[:], ti[:])
                return tf

            iota8f = iota_f32([128, 8], [[1, 8]])            # 0..7 per row
            c8x16 = iota_f32([8, 16], [[8, 16]])             # 0,8,...,120
            slotposf = iota_f32([8, NSLOT], [[8, NSLOT]])    # 0,8,...
            pidf = iota_f32([128, 1], [[0, 1]], cm=1)        # partition id
            iotaqf = iota_f32([128, NQ], [[1, NQ]])          # 0..NQ-1 per row
            tokid = cp.tile([128, 2], F32)                   # [t, 1]
            nc.vector.tensor_copy(tokid[:, 0:1], pidf[:])
            nc.vector.tensor_copy(tokid[:, 1:2], ones128[:])
            # per-partition bias tables for idx builds
            pmod16 = cp.tile([128, 1], F32)                  # p % 16
            for g in range(8):
                nc.sync.dma_start(pmod16[g * 16:(g + 1) * 16, :], pidf[0:16, :])
            epsb = cp.tile([128, 1], F32)
            nc.gpsimd.memset(epsb[:], float(EPS))

            # ================= weight prep (temps freed after) =================
            gwT = wp.tile([128, F_ * NCH, 16], F16)   # per c-chunk: 8 hi | 8 lo
            dwT = wp.tile([128, NCH, N, 64], F16)     # [h, hc, e, r]
            upwT = wp.tile([64, N, H], F16)
            with tc.tile_pool(name="prep", bufs=1) as pp, \
                 tc.tile_pool(name="prepps", bufs=2, space="PSUM") as pps:
                sig8 = pp.tile([8, 1], F32)
                for i in range(8):
                    nc.sync.dma_start(sig8[i:i + 1, :], sigma)
                GCH = C // 32
                gnorm2 = pp.tile([8, 32], F32)
                for q in range(32):
                    gchunk = pp.tile([8, GCH], F32, tag="gchunk")
                    nc.sync.dma_start(gchunk[:], gw[:, q * GCH:(q + 1) * GCH])
                    gsq = pp.tile([8, GCH], F32, tag="gsq")
                    nc.scalar.activation(gsq[:], gchunk[:], ACTF.Square,
                                         accum_out=gnorm2[:, q:q + 1])
                gn2 = pp.tile([8, 1], F32)
                nc.vector.reduce_sum(gn2[:], gnorm2[:], axis=AX.X)
                gnorm = pp.tile([8, 1], F32)
                nc.scalar.activation(gnorm[:], gn2[:], ACTF.Sqrt)
                ginv = pp.tile([8, 1], F32)
                nc.vector.reciprocal(ginv[:], gnorm[:])
                gscale = pp.tile([8, 1], F32)
                nc.vector.tensor_tensor(gscale[:], ginv[:], sig8[:], op=OP.mult)
                for q in range(32):
                    gchunk = pp.tile([8, GCH], F32, tag="gchunk")
                    nc.sync.dma_start(gchunk[:], gw[:, q * GCH:(q + 1) * GCH])
                    gwsc = pp.tile([8, GCH], F32, tag="gwsc")
                    nc.scalar.activation(gwsc[:], gchunk[:], ACTF.Copy, scale=gscale[:])
                    gwhi = pp.tile([8, GCH], F16, tag="gwhi")
                    nc.scalar.copy(gwhi[:], gwsc[:])
                    gwlo_f = pp.tile([8, GCH], F32, tag="gwlo_f")
                    nc.vector.tensor_tensor(gwlo_f[:], gwsc[:], gwhi[:], op=OP.subtract)
                    gwlo = pp.tile([8, GCH], F16, tag="gwlo")
                    nc.scalar.activation(gwlo[:], gwlo_f[:], ACTF.Copy, scale=1024.0)
                    for cc in range(GCH // 128):
                        ci = q * (GCH // 128) + cc
                        pt = pps.tile([128, 16], F16, tag="gwtp")
                        nc.tensor.transpose(pt[:, 0:8], gwhi[:, cc * 128:(cc + 1) * 128],
                                            identh[0:8, 0:8])
                        nc.tensor.transpose(pt[:, 8:16], gwlo[:, cc * 128:(cc + 1) * 128],
                                            identh[0:8, 0:8])
                        nc.scalar.copy(gwT[:, ci, :], pt[:])

                # transpose to dwT[h, hc, e, r] chunked by hc
                for ch in range(NCH):
                    dwf = pp.tile([128, 4, 128], F32, tag="dwf")
                    nc.sync.dma_start(dwf[:], dw[:, ch * 128:(ch + 1) * 128]
                                      .rearrange("(k p) h -> p k h", p=128))
                    dwn16 = pp.tile([128, 4, 128], F16, tag="dwn16")
                    nc.scalar.copy(dwn16[:], dwf[:])
                    for e in range(N):
                        g0 = e * 64
                        b0 = g0 % 128
                        pt3 = pps.tile([128, 64], F16, tag="dwtp")
                        nc.tensor.transpose(pt3[:], dwn16[b0:b0 + 64, g0 // 128, :],
                                            identh[b0:b0 + 64, b0:b0 + 64])
                        nc.scalar.copy(dwT[:, ch, e, :], pt3[:])

                for e in range(N):
                    uwe = pp.tile([128, NCH, 64], F32, tag="uwe")
                    nc.sync.dma_start(uwe[:], uw[e * H:(e + 1) * H, :]
                                      .rearrange("(ch p) r -> p ch r", p=128))
                    uwe16 = pp.tile([128, NCH, 64], F16, tag="uwe16")
                    nc.scalar.copy(uwe16[:], uwe[:])
                    for ch in range(NCH):
                        pt2 = pps.tile([64, 128], F16, tag="uwtp")
                        nc.tensor.transpose(pt2[:], uwe16[:, ch, :], identh[:])
                        nc.scalar.copy(upwT[0:64, e, ch * 128:(ch + 1) * 128], pt2[:])

            # ================= per-subtile main loop =================
            pstc = tc.tile_pool(name="pst", bufs=2, space="PSUM")
            psgc = tc.tile_pool(name="psg", bufs=1, space="PSUM")
            psmc = tc.tile_pool(name="psm", bufs=2, space="PSUM")
            psoc = tc.tile_pool(name="pso", bufs=2, space="PSUM")
            pst = pstc.__enter__()
            psg = psgc.__enter__()
            psm = psmc.__enter__()
            pso = psoc.__enter__()
            for st in range(NSUB):
                # ---- load + cast fp16
                nat16 = nat16p.tile([128, NBLK, H], F16)
                for j in range(NBLK):
                    natf = natp.tile([128, H], F32)
                    row0 = (st * NBLK + j) * 128
                    nc.sync.dma_start(natf[:], x[row0:row0 + 128, :])
                    nc.vector.tensor_copy(nat16[:, j, :], natf[:])

                # ---- transpose planes + gating + gram, hc-major
                logps = psg.tile([16, ST], F32, tag="logits")
                gram = psg.tile([128, 128], F32, tag="gram")
                for hc in range(NCH):
                    plane = planep.tile([128, NBLK * 128], F16)
                    for j4 in range(NBLK // 4):
                        pt = pst.tile([128, 512], F16, tag="xtp")
                        for jj in range(4):
                            j = j4 * 4 + jj
                            nc.tensor.transpose(pt[:, jj * 128:(jj + 1) * 128],
                                                nat16[:, j, hc * 128:(hc + 1) * 128],
                                                identh[:])
                        nc.scalar.copy(plane[:, j4 * 512:(j4 + 1) * 512], pt[:])
                    for f in range(F_):
                        ci = f * NCH + hc
                        first = (hc == 0 and f == 0)
                        last = (hc == NCH - 1 and f == F_ - 1)
                        sl = plane[:, f::F_]          # [128, 128 tokens]
                        nc.tensor.matmul(logps[:], gwT[:, ci, :], sl,
                                         start=first, stop=last)
                        nc.tensor.matmul(gram[:], sl, sl, start=first, stop=last)

                # ---- norms from gram diag
                gsb = sp.tile([128, 128], F32, tag="gsb")
                nc.vector.tensor_tensor(gsb[:], gram[:], diagmask[:], op=OP.mult)
                n2 = sp.tile([128, 1], F32, tag="n2")
                nc.vector.reduce_sum(n2[:], gsb[:], axis=AX.X)
                nrm = sp.tile([128, 1], F32, tag="nrm")
                nc.scalar.activation(nrm[:], n2[:], ACTF.Sqrt)
                xinv = sp.tile([128, 1], F32, tag="xinv")
                nc.vector.reciprocal(xinv[:], nrm[:])

                # ---- logits token-major
                lgsb = sp.tile([16, ST], F32, tag="lgsb")
                nc.scalar.copy(lgsb[:], logps[:])
                lgT_ps = psm.tile([128, 16], F32, tag="midps")
                nc.tensor.transpose(lgT_ps[:], lgsb[:], ident16[:])
                lgT = sp.tile([128, 16], F32, tag="lgTs")
                nc.vector.tensor_copy(lgT[:], lgT_ps[:])
                lg = sp.tile([128, 8], F32, tag="lg")
                nc.vector.tensor_scalar(lg[:], lgT[:, 8:16], 1.0 / 1024.0, None, op0=OP.mult)
                nc.vector.tensor_tensor(lg[:], lg[:], lgT[:, 0:8], op=OP.add)
                nc.vector.tensor_scalar(lg[:], lg[:], xinv[:], None, op0=OP.mult)

                # ---- gumbel + argmax
                ut = sp.tile([128, 8], F32, tag="ut")
                nc.sync.dma_start(ut[:], u[st * ST:(st + 1) * ST, :])
                ln1 = sp.tile([128, 8], F32, tag="ln1")
                nc.scalar.activation(ln1[:], ut[:], ACTF.Ln, bias=epsb[:], scale=1.0)
                ln2 = sp.tile([128, 8], F32, tag="ln2")
                nc.scalar.activation(ln2[:], ln1[:], ACTF.Ln, bias=epsb[:], scale=-1.0)
                y = sp.tile([128, 8], F32, tag="y")
                nc.vector.tensor_tensor(y[:], lg[:], ln2[:], op=OP.subtract)
                mx8 = sp.tile([128, 8], F32, tag="mx8")
                nc.vector.max(mx8[:], y[:])
                mi8 = sp.tile([128, 8], U32, tag="mi8")
                nc.vector.max_index(mi8[:], mx8[:], y[:])
                ef = sp.tile([128, 1], F32, tag="ef")
                nc.vector.tensor_copy(ef[:], mi8[:, 0:1])

                # ---- routing tables
                onehot = sp.tile([128, 8], F32, tag="onehot")
                nc.vector.tensor_scalar(onehot[:], iota8f[:], ef[:], None, op0=OP.is_equal)
                counts_ps = psm.tile([8, 1], F32, tag="midps")
                nc.tensor.matmul(counts_ps[:], onehot[:], ones128[:], start=True, stop=True)
                countsb = sp.tile([8, 1], F32, tag="countsb")
                nc.vector.tensor_copy(countsb[:], counts_ps[:])
                cgt = sp.tile([8, 16], F32, tag="cgt")
                nc.vector.tensor_scalar(cgt[:], c8x16[:], countsb[:], None, op0=OP.is_lt)
                cnt8 = sp.tile([8, 1], F32, tag="cnt8")
                nc.vector.reduce_sum(cnt8[:], cgt[:], axis=AX.X)
                nc.vector.tensor_scalar(cnt8[:], cnt8[:], 8.0, None, op0=OP.mult)
                off_ps = psm.tile([8, 1], F32, tag="midps")
                nc.tensor.matmul(off_ps[:], tri8[:], cnt8[:], start=True, stop=True)
                offsb = sp.tile([8, 1], F32, tag="offsb")
                nc.vector.tensor_copy(offsb[:], off_ps[:])
                rank_ps = psm.tile([128, 8], F32, tag="midps")
                nc.tensor.matmul(rank_ps[:], tril128[:], onehot[:], start=True, stop=True)
                rksel = sp.tile([128, 8], F32, tag="rksel")
                nc.vector.tensor_tensor(rksel[:], rank_ps[:], onehot[:], op=OP.mult)
                rank = sp.tile([128, 1], F32, tag="rank")
                nc.vector.reduce_sum(rank[:], rksel[:], axis=AX.X)
                ohT_ps = psm.tile([8, 128], F32, tag="midps")
                nc.tensor.transpose(ohT_ps[:], onehot[:], identf[:])
                ohT = sp.tile([8, 128], F32, tag="ohTs")
                nc.vector.tensor_copy(ohT[:], ohT_ps[:])
                pos_ps = psm.tile([128, 1], F32, tag="midps")
                nc.tensor.matmul(pos_ps[:], ohT[:], offsb[:], start=True, stop=True)
                pos = sp.tile([128, 1], F32, tag="pos")
                nc.vector.tensor_tensor(pos[:], pos_ps[:], rank[:], op=OP.add)
                # slot expert ids
                sge = sp.tile([8, NSLOT], F32, tag="sge")
                nc.vector.tensor_scalar(sge[:], slotposf[:], offsb[:], None, op0=OP.is_ge)
                se_ps = psm.tile([NSLOT, 1], F32, tag="midps")
                nc.tensor.matmul(se_ps[:], sge[:], ones128[0:8, :], start=True, stop=True)
                sef = sp.tile([NSLOT, 1], F32, tag="sef")
                nc.vector.tensor_scalar(sef[:], se_ps[:], -1.0, None, op0=OP.add)
                se32 = sp.tile([NSLOT, 1], I32, tag="se32")
                nc.vector.tensor_copy(se32[:], sef[:])
                # inverse permutation + pad marker
                pq = sp.tile([128, NQ], F32, tag="pq")
                nc.vector.tensor_scalar(pq[:], iotaqf[:], pos[:], None, op0=OP.is_equal)
                invm_ps = psm.tile([1, NQ], F32, tag="midps")
                nc.tensor.matmul(invm_ps[:], tokid[:, 0:1], pq[:], start=True, stop=True)
                inv = sp.tile([1, NQ], F32, tag="inv")
                nc.vector.tensor_copy(inv[:], invm_ps[:])
                hasm_ps = psm.tile([1, NQ], F32, tag="midps")
                nc.tensor.matmul(hasm_ps[:], tokid[:, 1:2], pq[:], start=True, stop=True)
                invb = sp.tile([1, NQ], F32, tag="invb")
                nc.vector.tensor_scalar(invb[:], hasm_ps[:], -BIGROW / 16.0,
                                        BIGROW / 16.0, op0=OP.mult, op1=OP.add)
                nc.vector.tensor_tensor(invb[:], invb[:], inv[:], op=OP.add)

                # ---- idx tables via ones-matmul broadcast + ACT scale/bias drains
                # x-gather idx: wrapped [p(f), q] = inv[q]*16 + p
                xgb_ps = psm.tile([32, NQ], F32, tag="midps")
                nc.tensor.matmul(xgb_ps[:], ones1x32[:], inv[:], start=True, stop=True)
                xg_f = sp.tile([32, NQ], F32, tag="xg_f")
                nc.scalar.activation(xg_f[:], xgb_ps[:], ACTF.Identity,
                                     bias=pmod16[0:32, :], scale=16.0)
                xgidx = sp.tile([128, NQ], I16, tag="xgidx")
                nc.vector.tensor_copy(xgidx[0:32, :], xg_f[:])
                for rep in range(1, 4):
                    nc.vector.tensor_copy(xgidx[rep * 32:(rep + 1) * 32, :], xgidx[0:32, :])
                # scatter rows table: scT [p=(q8,f), s] = invb[s*8+q8]*16 + f
                bv = sp.tile([8, NSLOT], F32, tag="bv")
                for q8 in range(8):
                    nc.sync.dma_start(bv[q8:q8 + 1, :], invb[:, q8::8])
                scb_ps = psm.tile([128, NSLOT], F32, tag="midps")
                nc.tensor.matmul(scb_ps[:], a16[:], bv[:], start=True, stop=True)
                scT_f = sp.tile([128, NSLOT], F32, tag="scT_f")
                nc.scalar.activation(scT_f[:], scb_ps[:], ACTF.Identity,
                                     bias=pmod16[:], scale=1.0)
                nc.vector.tensor_scalar(scT_f[:], scT_f[:], float(st * ST * F_), None,
                                        op0=OP.add)
                scT = sp.tile([128, NSLOT], I32, tag="scT")
                nc.vector.tensor_copy(scT[:], scT_f[:])

                # ---- gathers (transpose mode, SBUF source)
                G = 256
                sortxs = []
                goff = 0
                while goff < NSLOT * 128:
                    g = min(G, NSLOT * 128 - goff)
                    sx = sortp.tile([128, NCH, g], F16, tag=f"sortx{len(sortxs)}")
                    nc.gpsimd.dma_gather(
                        out_ap=sx[:],
                        in_ap=nat16[:].rearrange("p j h -> p (j h)"),
                        idxs_ap=xgidx[:, goff // 16:(goff + g) // 16],
                        num_idxs=g, num_idxs_reg=g,
                        elem_size=H, transpose=True,
                        sbuf_tokens_per_rank=128, sbuf_free_dim_per_rank=H * 2)
                    sortxs.append(sx)
                    goff += g

                def sortx_slice(hc, col0, ncols):
                    c = col0 // G
                    assert (col0 % G) + ncols <= G or True
                    return sortxs[c][:, hc, col0 - c * G:col0 - c * G + ncols]

                # ---- down (dynamic expert rhs) + mid transpose
                midT = midp.tile([64, NSLOT * 128], F16)
                evs = []
                for s in range(NSLOT):
                    ev = nc.values_load(se32[s:s + 1, 0:1], engines=[mybir.EngineType.PE],
                                        min_val=0, max_val=7, skip_runtime_bounds_check=True)
                    evs.append(ev)
                    mps = psm.tile([128, 64], F32, tag="midps")
                    for hc in range(NCH):
                        nc.tensor.matmul(mps[:], sortx_slice(hc, s * 128, 128),
                                         dwT[:, hc, bass.ds(ev, 1), :],
                                         start=(hc == 0), stop=(hc == NCH - 1))
                    mid16 = sp.tile([128, 64], F16, tag="mid16")
                    nc.vector.tensor_copy(mid16[:], mps[:])
                    mtp = psm.tile([64, 128], F16, tag="midps")
                    nc.tensor.transpose(mtp[:], mid16[:], identh[:])
                    nc.vector.tensor_copy(midT[:, s * 128:(s + 1) * 128], mtp[:])
                # ---- up + scatter out
                for s in range(NSLOT):
                    ev = evs[s]
                    osb = outp.tile([128, H], F32, tag="osb")
                    for j, w in ((0, 512), (1, 512), (2, 256)):
                        ops_t = pso.tile([128, 512], F32, tag="oups")
                        nc.tensor.matmul(ops_t[:, 0:w], midT[:, s * 128:(s + 1) * 128],
                                         upwT[:, bass.ds(ev, 1), j * 512:j * 512 + w],
                                         start=True, stop=True)
                        nc.scalar.copy(osb[:, j * 512:j * 512 + w], ops_t[:, 0:w])
                    nc.gpsimd.indirect_dma_start(
                        out=out, out_offset=bass.IndirectOffsetOnAxis(ap=scT[:, s:s + 1], axis=0),
                        in_=osb[:], in_offset=None,
                        bounds_check=BC * F_ - 1, oob_is_err=False)
            pso = psoc.__exit__(None, None, None)
            psm = psmc.__exit__(None, None, None)
            psg = psgc.__exit__(None, None, None)
            pst = pstc.__exit__(None, None, None)

    nc.compile()
    return nc


_NC_CACHE = {}


def kernel(x, u, gate_w, sigma, down_w, up_w):
    if "nc" not in _NC_CACHE:
        _NC_CACHE["nc"] = build_nc()
    nc = _NC_CACHE["nc"]
    in_maps = []
    for c in range(NCORE):
        in_maps.append({
            "x": np.ascontiguousarray(np.asarray(x[c * BC:(c + 1) * BC], np.float32).reshape(BC * F_, H)),
            "u": np.ascontiguousarray(np.asarray(u[c * BC:(c + 1) * BC], np.float32)),
            "gate_w": np.ascontiguousarray(np.asarray(gate_w, np.float32)),
            "sigma": np.asarray(sigma, np.float32).reshape(1, 1),
            "down_w": np.ascontiguousarray(np.asarray(down_w, np.float32).reshape(N * R, H)),
            "up_w": np.ascontiguousarray(np.asarray(up_w, np.float32).reshape(N * H, R)),
        })
    res = run_bass_kernel_spmd(nc, in_maps, core_ids=list(range(NCORE)))
    outs = [r["out"].reshape(BC, F_, H) for r in res.results]
    return np.concatenate(outs, axis=0)
